# revision 14
# baseline (speedup 1.0000x reference)
"""DNC (Differentiable Neural Computer) sequential kernel for Trainium2.

Single-core Bass/Tile implementation: the 8192-step recurrence is strictly
sequential (sharding hint: unshardable within one sequence), so we run the
whole scan on core 0 with a hardware For_i loop, keeping all state in SBUF.

Key design points:
- ACT table set 6 (natural_log_exp_and_others) preloaded once; every
  transcendental (sigmoid/tanh/softplus/sqrt/softmax-exp) is expressed via
  exp/ln so no table switches ever happen.
- sigmoid(x) = 1/(1+e^-x); tanh(x) = 1-2/(1+e^{2x}); per-gate input signs
  and scales are folded into the weights on the host.
- x-projection of LSTM0 (W_ih0[:, :10] @ x_t) is precomputed on the host
  for all t (batched), streamed in per-chunk via DMA.
- The output projection is folded: y_t = (W_fc @ W_out) @ [h1; rv] + b,
  computed per-step on the PE and streamed out per-chunk.
- All tiny contractions / broadcasts / transposes run on the PE via
  matmuls with ones-vectors and identity; elementwise on DVE; exp/ln on ACT.
- Allocation weighting is sort-free: pairwise comparisons + masked cumprod
  (tensor_tensor_scan) reproduce the stable argsort semantics exactly.
"""

import sys
import os
import numpy as np

if "/opt/trn_rl_repo" not in sys.path:
    sys.path.insert(0, "/opt/trn_rl_repo")

# DNC hyperparameters (fixed by the problem)
N, CELL, R, H, X_DIM, OUT_DIM = 10, 20, 1, 128, 10, 10
IFACE = 88
EPS, DELTA = 1e-6, 1e-6
SEQ_LEN = 8192


def build(T=SEQ_LEN, U=8):
    """Build the Bass program. Returns (nc, names dict)."""
    import concourse.bass as bass
    import concourse.bacc as bacc
    from concourse import bass_isa
    import concourse.mybir as mybir
    from concourse import tile

    F32 = mybir.dt.float32
    F16 = mybir.dt.float16 if os.environ.get("CTRL_F16", "0") == "1" else mybir.dt.float32
    AF = mybir.ActivationFunctionType
    OP = mybir.AluOpType
    ds = bass.ds

    assert T % U == 0

    nc = bacc.Bacc(None, target_bir_lowering=False, debug=False)
    names = {}

    with tile.TileContext(nc) as tc:
        with tc.tile_pool(name="dram", bufs=1, space="DRAM") as dram, \
             tc.tile_pool(name="consts", bufs=1) as consts, \
             tc.tile_pool(name="state", bufs=1) as state, \
             tc.tile_pool(name="work", bufs=2) as work, \
             tc.tile_pool(name="io", bufs=2) as io, \
             tc.tile_pool(name="psG", bufs=2, space="PSUM") as psG, \
             tc.tile_pool(name="psX", bufs=1, space="PSUM") as psX, \
             tc.tile_pool(name="psA", bufs=1, space="PSUM") as psA, \
             tc.tile_pool(name="psB", bufs=1, space="PSUM") as psB, \
             tc.tile_pool(name="psC", bufs=1, space="PSUM") as psC:

            # ---------------- DRAM I/O ----------------
            d_xb3 = dram.tile([128, T, 4], F32, kind="ExternalInput")
            d_w0ht = dram.tile([128, 512], F16, kind="ExternalInput")
            d_w0rt = dram.tile([20, 512], F16, kind="ExternalInput")
            d_w1it = dram.tile([128, 512], F16, kind="ExternalInput")
            d_w1ht = dram.tile([128, 512], F16, kind="ExternalInput")
            d_b1 = dram.tile([128, 4], F32, kind="ExternalInput")
            d_wxit = dram.tile([128, 100], F16, kind="ExternalInput")
            d_wpit = dram.tile([128, 3], F16, kind="ExternalInput")
            d_wct1 = dram.tile([128, 10], F16, kind="ExternalInput")
            d_wct2 = dram.tile([20, 10], F16, kind="ExternalInput")
            d_bcomb = dram.tile([10, 1], F32, kind="ExternalInput")
            d_idn = dram.tile([128, 128], F32, kind="ExternalInput")
            d_jlt = dram.tile([10, 10], F32, kind="ExternalInput")
            d_diagm = dram.tile([10, 10], F32, kind="ExternalInput")
            d_yt = dram.tile([10, T], F32, kind="ExternalOutput")
            for k, v in [("xb3", d_xb3), ("w0ht", d_w0ht), ("w0rt", d_w0rt),
                         ("w1it", d_w1it), ("w1ht", d_w1ht), ("b1", d_b1),
                         ("wxit", d_wxit), ("wpit", d_wpit),
                         ("wct1", d_wct1), ("wct2", d_wct2), ("bcomb", d_bcomb),
                         ("idn", d_idn), ("jlt", d_jlt), ("diagm", d_diagm),
                         ("yt", d_yt)]:
                names[k] = v.tensor.name

            # ---- preload ACT function set 6 = natural_log_exp_and_others ----
            if os.environ.get("NO_PRELOAD", "0") != "1":
                nc.scalar.add_instruction(mybir.InstLoadActFuncSet(
                    name=nc.get_next_instruction_name(), act_func_set_id=6,
                    ins=[], outs=[]))

            # ---------------- const SBUF tiles ----------------
            W0hT = consts.tile([128, 512], F16)
            W0rT = consts.tile([20, 512], F16)
            W1iT = consts.tile([128, 512], F16)
            W1hT = consts.tile([128, 512], F16)
            B1 = consts.tile([128, 4], F32)
            WxiT = consts.tile([128, 100], F16)
            WpiT = consts.tile([128, 3], F16)
            WcT1 = consts.tile([128, 10], F16)
            WcT2 = consts.tile([20, 10], F16)
            BCOMB = consts.tile([10, 1], F32)
            IDN = consts.tile([128, 128], F32)
            JLT = consts.tile([10, 10], F32)
            DIAGM = consts.tile([10, 10], F32)
            ONESR = consts.tile([1, 32], F32)
            ONESC = consts.tile([32, 1], F32)
            ONES2D = consts.tile([10, 10], F32)

            for dst, src in [(W0hT, d_w0ht), (W0rT, d_w0rt), (W1iT, d_w1it),
                             (W1hT, d_w1ht), (B1, d_b1), (WxiT, d_wxit),
                             (WpiT, d_wpit), (WcT1, d_wct1),
                             (WcT2, d_wct2), (BCOMB, d_bcomb), (IDN, d_idn),
                             (JLT, d_jlt), (DIAGM, d_diagm)]:
                nc.sync.dma_start(dst[:], src[:])
            nc.vector.memset(ONESR[:], 1.0)
            nc.vector.memset(ONESC[:], 1.0)
            nc.vector.memset(ONES2D[:], 1.0)

            # ---------------- state tiles ----------------
            h0 = state.tile([128, 1], F16)
            c0 = state.tile([128, 1], F32)
            h1 = state.tile([128, 1], F16)
            c1 = state.tile([128, 1], F32)
            rv = state.tile([20, 1], F16)
            M = state.tile([10, 20], F32)
            Mt = state.tile([20, 10], F32)
            L = state.tile([10, 10], F32)
            LT = state.tile([10, 10], F32)
            negu = state.tile([1, 10], F32)     # -u
            p_row = state.tile([1, 10], F32)
            ww_row = state.tile([1, 10], F32)
            rw_row = state.tile([1, 10], F32)
            rw_col = state.tile([10, 1], F32)
            zM = state.tile([10, 1], F32)       # ||M_i||^2
            scr = state.tile([10, 20], F32)     # ttr dummy out

            for t in (h0, c0, h1, c1, rv, M, Mt, L, LT, negu, p_row,
                      ww_row, rw_row, rw_col, zM, scr):
                nc.vector.memset(t[:], 0.0)

            # =========================================================

            USE_GP = os.environ.get("NO_GPSIMD", "0") != "1"

            def bcast10(dst_sb, src_row_ap, ps_tile, ps_col):
                """broadcast [1,k]@p0 -> [10,k] SBUF tile"""
                if USE_GP:
                    nc.gpsimd.partition_broadcast(dst_sb[:], src_row_ap)
                else:
                    k = dst_sb.shape[1]
                    nc.tensor.matmul(ps_tile[0:10, ps_col:ps_col + k],
                                     lhsT=ONESR[:, 0:10], rhs=src_row_ap,
                                     start=True, stop=True)
                    nc.vector.tensor_copy(dst_sb[:],
                                          ps_tile[0:10, ps_col:ps_col + k])

            def sum10(dst_sb, src_col, ps_tile, ps_col):
                """dst[0:1,0:1] = sum over 10 partitions of src_col [10,1]"""
                if USE_GP:
                    nc.gpsimd.partition_all_reduce(
                        dst_sb[:], src_col[:], channels=10,
                        reduce_op=bass_isa.ReduceOp.add)
                else:
                    nc.tensor.matmul(ps_tile[0:1, ps_col:ps_col + 1],
                                     lhsT=src_col[:], rhs=ONESC[0:10, :],
                                     start=True, stop=True)
                    nc.vector.tensor_copy(dst_sb[0:1, 0:1],
                                          ps_tile[0:1, ps_col:ps_col + 1])
            def lstm_cell(G, WaT, rhs_a, WbT, rhs_b, xb_ap, c, h, tag):
                """gates psum <- sum of 8 matmuls; then exp-based cell."""
                for g in range(4):
                    nc.tensor.matmul(G[:, g:g + 1],
                                     lhsT=WaT[:, 128 * g:128 * (g + 1)],
                                     rhs=rhs_a[:], start=True, stop=False)
                    nc.tensor.matmul(G[:, g:g + 1],
                                     lhsT=WbT[:, 128 * g:128 * (g + 1)],
                                     rhs=rhs_b[:], start=False, stop=True)
                z = work.tile([128, 4], F32, tag=f"z{tag}")
                nc.vector.tensor_tensor(out=z[:], in0=G[:], in1=xb_ap,
                                        op=OP.add)
                E = work.tile([128, 4], F32, tag=f"E{tag}")
                nc.scalar.activation(E[:], z[:], AF.Exp)
                Rr = work.tile([128, 4], F32, tag=f"R{tag}")
                nc.vector.tensor_scalar(out=Rr[:], in0=E[:], scalar1=1.0,
                                        scalar2=None, op0=OP.add)
                nc.vector.reciprocal(Rr[:], Rr[:])
                m_ = work.tile([128, 1], F32, tag=f"m{tag}")
                nc.vector.tensor_tensor(out=m_[:], in0=Rr[:, 0:1],
                                        in1=Rr[:, 2:3], op=OP.mult)
                A_ = work.tile([128, 1], F32, tag=f"A{tag}")
                nc.vector.scalar_tensor_tensor(
                    out=A_[:], in0=m_[:], scalar=-2.0, in1=Rr[:, 0:1],
                    op0=OP.mult, op1=OP.add)
                nc.vector.scalar_tensor_tensor(
                    out=c[:], in0=c[:], scalar=Rr[:, 1:2], in1=A_[:],
                    op0=OP.mult, op1=OP.add)
                E2 = work.tile([128, 1], F32, tag=f"E2{tag}")
                nc.scalar.activation(E2[:], c[:], AF.Exp, scale=2.0)
                R2 = work.tile([128, 1], F32, tag=f"R2{tag}")
                nc.vector.tensor_scalar(out=R2[:], in0=E2[:], scalar1=1.0,
                                        scalar2=None, op0=OP.add)
                nc.vector.reciprocal(R2[:], R2[:])
                m2 = work.tile([128, 1], F32, tag=f"m2{tag}")
                nc.vector.tensor_tensor(out=m2[:], in0=Rr[:, 3:4], in1=R2[:],
                                        op=OP.mult)
                nc.vector.scalar_tensor_tensor(
                    out=h[:], in0=m2[:], scalar=-2.0, in1=Rr[:, 3:4],
                    op0=OP.mult, op1=OP.add)

            SKIP_MEM = os.environ.get("SKIP_MEM", "0") == "1"
            SKIP_CTRL = os.environ.get("SKIP_CTRL", "0") == "1"

            def step(XB, YT, u):
                # ---------- controller ----------
                if not SKIP_CTRL:
                    G0 = psG.tile([128, 4], F32, tag="G")
                    lstm_cell(G0, W0hT, h0, W0rT, rv, XB[:, u, :], c0, h0, "0")
                    G1 = psG.tile([128, 4], F32, tag="G")
                    lstm_cell(G1, W1iT, h0, W1hT, h1, B1[:], c1, h1, "1")

                # ---------- interface xi ----------
                # 5 segment matmuls, each landing at base partition 0 in its
                # own PSUM column: col0=rk col1=wk col2=wv col3=er
                # col4=[fg,ga,gw,rb,wb] (rows 0:5). pi -> row [1,3] cols 5:8.
                XIPI = psX.tile([32, 8], F32, tag="XIPI")
                for s in range(5):
                    nc.tensor.matmul(XIPI[0:20, s:s + 1],
                                     lhsT=WxiT[:, 20 * s:20 * (s + 1)],
                                     rhs=h1[:], start=True, stop=True)
                nc.tensor.matmul(XIPI[0:1, 5:8], lhsT=h1[:], rhs=WpiT[:],
                                 start=True, stop=True)
                # E = exp(scaled pre-acts); D = 1+E; R = 1/D
                DX = work.tile([20, 5], F32, tag="DX")
                nc.scalar.activation(DX[:], XIPI[0:20, 0:5], AF.Exp)
                nc.vector.tensor_scalar(out=DX[:], in0=DX[:], scalar1=1.0,
                                        scalar2=None, op0=OP.add)
                XIV = work.tile([20, 5], F32, tag="XIV")
                nc.vector.reciprocal(XIV[:], DX[:])
                # tanh groups (rk, wk, wv): t = 1-2R
                nc.vector.tensor_scalar(out=XIV[:, 0:3], in0=XIV[:, 0:3],
                                        scalar1=-2.0, scalar2=1.0,
                                        op0=OP.mult, op1=OP.add)
                # softplus rows (rb, wb) in col 4 rows 3:5: ln(1+e^x)
                nc.scalar.activation(XIV[0:2, 4:5], DX[0:2, 4:5], AF.Ln)
                # pi softmax (row form)
                EP = work.tile([1, 3], F32, tag="EP")
                sEP = work.tile([1, 1], F32, tag="sEP")
                nc.scalar.activation(EP[:], XIPI[0:1, 5:8], AF.Exp,
                                     accum_out=sEP[:])
                rsp = work.tile([1, 1], F32, tag="rsp")
                nc.vector.reciprocal(rsp[:], sEP[:])
                PI = work.tile([1, 3], F32, tag="PI")
                nc.vector.tensor_scalar(out=PI[:], in0=EP[:], scalar1=rsp[:],
                                        scalar2=None, op0=OP.mult)

                # transposes: wv col, er col, scal5 col -> one SBUF row
                spC = psC.tile([32, 256], F32, tag="spC")
                nc.tensor.transpose(spC[0:1, 0:20], XIV[:, 2:3],
                                    IDN[0:20, 0:20])
                nc.tensor.transpose(spC[0:1, 20:40], XIV[:, 3:4],
                                    IDN[0:20, 0:20])
                nc.tensor.transpose(spC[0:1, 40:45], XIV[0:5, 4:5],
                                    IDN[0:5, 0:5])
                ERWV = work.tile([1, 45], F32, tag="ERWV")
                nc.vector.tensor_copy(ERWV[:], spC[0:1, 0:45])
                # scal row = [rb, wb, fg, ga, gw]; ERWV[:, 0:40] = [wv | er]

                if SKIP_MEM:
                    nc.tensor.matmul(spC[0:10, 138:139], lhsT=WcT1[:],
                                     rhs=h1[:], start=True, stop=False)
                    nc.tensor.matmul(spC[0:10, 138:139], lhsT=WcT2[:],
                                     rhs=rv[:], start=False, stop=True)
                    nc.vector.tensor_scalar(out=YT[:, u:u + 1],
                                            in0=spC[0:10, 138:139],
                                            scalar1=BCOMB[:], scalar2=None,
                                            op0=OP.add)
                    return
                # ---------- usage / allocation ----------
                psiN = work.tile([1, 10], F32, tag="psiN")
                nc.vector.scalar_tensor_tensor(
                    out=psiN[:], in0=rw_row[:], scalar=ERWV[:, 42:43],
                    in1=ONESR[:, 0:10], op0=OP.mult, op1=OP.subtract)
                qq = work.tile([1, 10], F32, tag="qq")
                nc.vector.tensor_scalar(out=qq[:], in0=negu[:], scalar1=1.0,
                                        scalar2=None, op0=OP.add)
                rr = work.tile([1, 10], F32, tag="rr")
                nc.vector.tensor_tensor(out=rr[:], in0=ww_row[:], in1=qq[:],
                                        op=OP.mult)
                ss = work.tile([1, 10], F32, tag="ss")
                nc.vector.tensor_tensor(out=ss[:], in0=rr[:], in1=negu[:],
                                        op=OP.subtract)
                nc.vector.tensor_tensor(out=negu[:], in0=ss[:], in1=psiN[:],
                                        op=OP.mult)
                ue = work.tile([1, 10], F32, tag="ue")
                nc.vector.tensor_scalar(out=ue[:], in0=negu[:],
                                        scalar1=-(1.0 - DELTA), scalar2=DELTA,
                                        op0=OP.mult, op1=OP.add)

                spA = psA.tile([32, 256], F32, tag="spA")
                nc.tensor.transpose(spA[0:10, 0:1], ue[:], IDN[0:1, 0:1])
                nc.tensor.matmul(spA[0:10, 1:11], lhsT=ONESR[:, 0:10],
                                 rhs=ue[:], start=True, stop=True)
                uec = work.tile([10, 1], F32, tag="uec")
                nc.vector.tensor_copy(uec[:], spA[0:10, 0:1])
                Acmp = work.tile([10, 10], F32, tag="Acmp")
                nc.vector.tensor_scalar(out=Acmp[:], in0=spA[0:10, 1:11],
                                        scalar1=uec[:], scalar2=None,
                                        op0=OP.is_lt)
                Bcmp = work.tile([10, 10], F32, tag="Bcmp")
                nc.vector.scalar_tensor_tensor(
                    out=Bcmp[:], in0=spA[0:10, 1:11], scalar=uec[:],
                    in1=JLT[:], op0=OP.is_equal, op1=OP.mult)
                less = work.tile([10, 10], F32, tag="less")
                nc.vector.tensor_tensor(out=less[:], in0=Acmp[:], in1=Bcmp[:],
                                        op=OP.add)
                UEm1 = work.tile([10, 10], F32, tag="UEm1")
                nc.vector.tensor_scalar(out=UEm1[:], in0=spA[0:10, 1:11],
                                        scalar1=-1.0, scalar2=None, op0=OP.add)
                sel = work.tile([10, 10], F32, tag="sel")
                nc.vector.tensor_tensor(out=sel[:], in0=less[:], in1=UEm1[:],
                                        op=OP.mult)
                nc.vector.tensor_scalar(out=sel[:], in0=sel[:], scalar1=1.0,
                                        scalar2=None, op0=OP.add)
                cpv = work.tile([10, 10], F32, tag="cpv")
                nc.vector.tensor_tensor_scan(out=cpv[:], data0=sel[:],
                                             data1=ONES2D[:], initial=1.0,
                                             op0=OP.mult, op1=OP.mult)
                negalloc = work.tile([10, 1], F32, tag="negalloc")
                nc.vector.scalar_tensor_tensor(
                    out=negalloc[:], in0=uec[:], scalar=cpv[:, 9:10],
                    in1=cpv[:, 9:10], op0=OP.mult, op1=OP.subtract)

                # ---------- content weight (write key) ----------
                spB = psB.tile([32, 256], F32, tag="spB")
                nc.tensor.matmul(spB[0:10, 0:1], lhsT=Mt[:], rhs=XIV[0:20, 1:2],
                                 start=True, stop=True)
                nc.tensor.matmul(spB[0:1, 1:2], lhsT=XIV[0:20, 1:2],
                                 rhs=XIV[0:20, 1:2], start=True, stop=True)
                zks = work.tile([1, 1], F32, tag="zks")
                nc.vector.tensor_copy(zks[:], spB[0:1, 1:2])
                zkb = work.tile([10, 1], F32, tag="zkb")
                bcast10(zkb, zks[:], spB, 2)
                qw = work.tile([10, 1], F32, tag="qw")
                nc.vector.tensor_scalar(out=qw[:], in0=zM[:],
                                        scalar1=zkb[:], scalar2=1e-38,
                                        op0=OP.mult, op1=OP.max)
                nc.scalar.activation(qw[:], qw[:], AF.Ln)
                nc.scalar.activation(qw[:], qw[:], AF.Exp, scale=0.5)
                nc.vector.tensor_scalar(out=qw[:], in0=qw[:], scalar1=EPS,
                                        scalar2=None, op0=OP.add)
                rdd = work.tile([10, 1], F32, tag="rdd")
                nc.vector.reciprocal(rdd[:], qw[:])
                sim_ = work.tile([10, 1], F32, tag="sim_")
                nc.vector.tensor_tensor(out=sim_[:], in0=spB[0:10, 0:1],
                                        in1=rdd[:], op=OP.mult)
                bb = work.tile([10, 1], F32, tag="bb")
                bcast10(bb, ERWV[:, 41:42], spB, 3)
                bs = work.tile([10, 1], F32, tag="bs")
                nc.vector.tensor_tensor(out=bs[:], in0=sim_[:],
                                        in1=bb[:], op=OP.mult)
                ew = work.tile([10, 1], F32, tag="ew")
                nc.scalar.activation(ew[:], bs[:], AF.Exp)
                nc.tensor.matmul(spB[0:1, 4:5], lhsT=ew[:], rhs=ONESC[0:10, :],
                                 start=True, stop=True)
                rsw = work.tile([1, 1], F32, tag="rsw")
                nc.vector.reciprocal(rsw[:], spB[0:1, 4:5])
                nc.tensor.matmul(spB[0:10, 5:6], lhsT=ONESR[:, 0:10],
                                 rhs=rsw[:], start=True, stop=True)
                cw = work.tile([10, 1], F32, tag="cw")
                nc.vector.tensor_tensor(out=cw[:], in0=ew[:],
                                        in1=spB[0:10, 5:6], op=OP.mult)

                # ---------- write weights ----------
                mga = work.tile([1, 1], F32, tag="mga")
                nc.vector.tensor_tensor(out=mga[:], in0=ERWV[:, 43:44],
                                        in1=ERWV[:, 44:45], op=OP.mult)
                CO = work.tile([1, 2], F32, tag="CO")
                nc.vector.tensor_scalar(out=CO[:, 0:1], in0=mga[:],
                                        scalar1=-1.0, scalar2=None,
                                        op0=OP.mult)
                nc.vector.scalar_tensor_tensor(
                    out=CO[:, 1:2], in0=mga[:], scalar=-1.0, in1=ERWV[:, 44:45],
                    op0=OP.mult, op1=OP.add)
                cob = work.tile([10, 2], F32, tag="cob")
                bcast10(cob, CO[:], spB, 6)
                t2 = work.tile([10, 1], F32, tag="t2")
                nc.vector.tensor_tensor(out=t2[:], in0=cw[:],
                                        in1=cob[:, 1:2], op=OP.mult)
                wwc = work.tile([10, 1], F32, tag="wwc")
                nc.vector.scalar_tensor_tensor(
                    out=wwc[:], in0=negalloc[:], scalar=cob[:, 0:1],
                    in1=t2[:], op0=OP.mult, op1=OP.add)
                nc.tensor.transpose(spB[0:1, 8:18], wwc[:], IDN[0:10, 0:10])
                nc.vector.tensor_copy(ww_row[:], spB[0:1, 8:18])

                # ---------- memory update ----------
                nc.tensor.matmul(spC[0:10, 45:85], lhsT=ONESR[:, 0:10],
                                 rhs=ERWV[:, 0:40], start=True, stop=True)
                m1 = work.tile([10, 20], F32, tag="m1")
                nc.vector.scalar_tensor_tensor(
                    out=m1[:], in0=spC[0:10, 65:85], scalar=wwc[:], in1=M[:],
                    op0=OP.mult, op1=OP.mult)
                M2 = work.tile([10, 20], F32, tag="M2")
                nc.vector.tensor_tensor(out=M2[:], in0=M[:], in1=m1[:],
                                        op=OP.subtract)
                nc.vector.scalar_tensor_tensor(
                    out=M[:], in0=spC[0:10, 45:65], scalar=wwc[:], in1=M2[:],
                    op0=OP.mult, op1=OP.add)
                nc.vector.scalar_tensor_tensor(
                    out=scr[:], in0=M[:], scalar=1.0, in1=M[:],
                    op0=OP.mult, op1=OP.mult, accum_out=zM[:])
                nc.tensor.matmul(spC[0:20, 85:95], lhsT=ONESR[:, 0:20],
                                 rhs=ww_row[:], start=True, stop=True)
                m1t = work.tile([20, 10], F32, tag="m1t")
                nc.vector.scalar_tensor_tensor(
                    out=m1t[:], in0=spC[0:20, 85:95], scalar=XIV[0:20, 3:4],
                    in1=Mt[:], op0=OP.mult, op1=OP.mult)
                Mt2 = work.tile([20, 10], F32, tag="Mt2")
                nc.vector.tensor_tensor(out=Mt2[:], in0=Mt[:], in1=m1t[:],
                                        op=OP.subtract)
                nc.vector.scalar_tensor_tensor(
                    out=Mt[:], in0=spC[0:20, 85:95], scalar=XIV[0:20, 2:3],
                    in1=Mt2[:], op0=OP.mult, op1=OP.add)

                # ---------- link matrix ----------
                nc.tensor.matmul(spC[0:10, 95:105], lhsT=ONESR[:, 0:10],
                                 rhs=ww_row[:], start=True, stop=True)
                nc.tensor.matmul(spC[0:10, 105:115], lhsT=ONESR[:, 0:10],
                                 rhs=p_row[:], start=True, stop=True)
                S_ = work.tile([10, 10], F32, tag="S_")
                nc.vector.scalar_tensor_tensor(
                    out=S_[:], in0=spC[0:10, 95:105], scalar=wwc[:], in1=L[:],
                    op0=OP.add, op1=OP.mult)
                LmS = work.tile([10, 10], F32, tag="LmS")
                nc.vector.tensor_tensor(out=LmS[:], in0=L[:], in1=S_[:],
                                        op=OP.subtract)
                Ln_ = work.tile([10, 10], F32, tag="Ln_")
                nc.vector.scalar_tensor_tensor(
                    out=Ln_[:], in0=spC[0:10, 105:115], scalar=wwc[:],
                    in1=LmS[:], op0=OP.mult, op1=OP.add)
                nc.vector.tensor_tensor(out=L[:], in0=Ln_[:], in1=DIAGM[:],
                                        op=OP.mult)
                S2 = work.tile([10, 10], F32, tag="S2")
                nc.vector.scalar_tensor_tensor(
                    out=S2[:], in0=spC[0:10, 95:105], scalar=wwc[:], in1=LT[:],
                    op0=OP.add, op1=OP.mult)
                LmS2 = work.tile([10, 10], F32, tag="LmS2")
                nc.vector.tensor_tensor(out=LmS2[:], in0=LT[:], in1=S2[:],
                                        op=OP.subtract)
                Lt_ = work.tile([10, 10], F32, tag="Lt_")
                nc.vector.scalar_tensor_tensor(
                    out=Lt_[:], in0=spC[0:10, 105:115], scalar=wwc[:],
                    in1=LmS2[:], op0=OP.mult, op1=OP.add)
                nc.vector.tensor_tensor(out=LT[:], in0=Lt_[:], in1=DIAGM[:],
                                        op=OP.mult)

                # ---------- precedence ----------
                swp = work.tile([10, 1], F32, tag="swp")
                sum10(swp, wwc, spC, 115)
                tp = work.tile([1, 10], F32, tag="tp")
                nc.vector.scalar_tensor_tensor(
                    out=tp[:], in0=p_row[:], scalar=swp[0:1, 0:1],
                    in1=ww_row[:], op0=OP.mult, op1=OP.subtract)
                nc.vector.tensor_tensor(out=p_row[:], in0=p_row[:], in1=tp[:],
                                        op=OP.subtract)

                # ---------- read ----------
                nc.tensor.matmul(spC[0:1, 116:126], lhsT=rw_col[:], rhs=L[:],
                                 start=True, stop=True)     # bwd
                nc.tensor.matmul(spC[0:1, 126:136], lhsT=rw_col[:], rhs=LT[:],
                                 start=True, stop=True)     # fwd
                nc.tensor.matmul(spB[0:10, 20:21], lhsT=Mt[:], rhs=XIV[0:20, 0:1],
                                 start=True, stop=True)     # dotr
                nc.tensor.matmul(spB[0:1, 21:22], lhsT=XIV[0:20, 0:1],
                                 rhs=XIV[0:20, 0:1], start=True, stop=True)
                zkrs = work.tile([1, 1], F32, tag="zkrs")
                nc.vector.tensor_copy(zkrs[:], spB[0:1, 21:22])
                zkrb = work.tile([10, 1], F32, tag="zkrb")
                bcast10(zkrb, zkrs[:], spB, 22)
                qr = work.tile([10, 1], F32, tag="qr")
                nc.vector.tensor_scalar(out=qr[:], in0=zM[:],
                                        scalar1=zkrb[:],
                                        scalar2=1e-38, op0=OP.mult, op1=OP.max)
                nc.scalar.activation(qr[:], qr[:], AF.Ln)
                nc.scalar.activation(qr[:], qr[:], AF.Exp, scale=0.5)
                nc.vector.tensor_scalar(out=qr[:], in0=qr[:], scalar1=EPS,
                                        scalar2=None, op0=OP.add)
                rdr = work.tile([10, 1], F32, tag="rdr")
                nc.vector.reciprocal(rdr[:], qr[:])
                simr = work.tile([10, 1], F32, tag="simr")
                nc.vector.tensor_tensor(out=simr[:], in0=spB[0:10, 20:21],
                                        in1=rdr[:], op=OP.mult)
                rbb = work.tile([10, 1], F32, tag="rbb")
                bcast10(rbb, ERWV[:, 40:41], spB, 23)
                bsr = work.tile([10, 1], F32, tag="bsr")
                nc.vector.tensor_tensor(out=bsr[:], in0=simr[:],
                                        in1=rbb[:], op=OP.mult)
                er_ = work.tile([10, 1], F32, tag="er_")
                nc.scalar.activation(er_[:], bsr[:], AF.Exp)
                nc.tensor.matmul(spB[0:1, 24:25], lhsT=er_[:],
                                 rhs=ONESC[0:10, :], start=True, stop=True)
                rswr = work.tile([1, 1], F32, tag="rswr")
                nc.vector.reciprocal(rswr[:], spB[0:1, 24:25])
                nc.tensor.matmul(spB[0:10, 25:26], lhsT=ONESR[:, 0:10],
                                 rhs=rswr[:], start=True, stop=True)
                crc = work.tile([10, 1], F32, tag="crc")
                nc.vector.tensor_tensor(out=crc[:], in0=er_[:],
                                        in1=spB[0:10, 25:26], op=OP.mult)
                nc.tensor.transpose(spB[0:1, 26:36], crc[:], IDN[0:10, 0:10])

                Bf = work.tile([1, 10], F32, tag="Bf")
                nc.vector.tensor_scalar(out=Bf[:], in0=spC[0:1, 126:136],
                                        scalar1=PI[:, 2:3], scalar2=None,
                                        op0=OP.mult)
                Af = work.tile([1, 10], F32, tag="Af")
                nc.vector.scalar_tensor_tensor(
                    out=Af[:], in0=spC[0:1, 116:126], scalar=PI[:, 0:1],
                    in1=Bf[:], op0=OP.mult, op1=OP.add)
                nc.vector.scalar_tensor_tensor(
                    out=rw_row[:], in0=spB[0:1, 26:36], scalar=PI[:, 1:2],
                    in1=Af[:], op0=OP.mult, op1=OP.add)
                nc.tensor.transpose(spC[0:10, 136:137], rw_row[:],
                                    IDN[0:1, 0:1])
                nc.vector.tensor_copy(rw_col[:], spC[0:10, 136:137])
                nc.tensor.matmul(spC[0:20, 137:138], lhsT=M[:], rhs=rw_col[:],
                                 start=True, stop=True)
                nc.vector.tensor_copy(rv[:], spC[0:20, 137:138])

                # ---------- output ----------
                nc.tensor.matmul(spC[0:10, 138:139], lhsT=WcT1[:], rhs=h1[:],
                                 start=True, stop=False)
                nc.tensor.matmul(spC[0:10, 138:139], lhsT=WcT2[:], rhs=rv[:],
                                 start=False, stop=True)
                nc.vector.tensor_scalar(out=YT[:, u:u + 1],
                                        in0=spC[0:10, 138:139],
                                        scalar1=BCOMB[:], scalar2=None,
                                        op0=OP.add)

            # ================= main loop =================
            import concourse.mybir as _mb
            _hints = () if os.environ.get("NO_HINTS", "0") == "1" else tuple(_mb.ALL_ENGINES)
            with tc.For_i(0, T, U, hint_engines=_hints) as iv:
                XB = io.tile([128, U, 4], F32, tag="XB")
                nc.sync.dma_start(XB[:], d_xb3[:, ds(iv, U), :])
                YT = io.tile([10, U], F32, tag="YT")
                for u in range(U):
                    step(XB, YT, u)
                nc.sync.dma_start(d_yt[:, ds(iv, U)], YT[:])

    nc.compile()

    # sanity: exactly one ACT table load (the manual preload)
    n_loads = sum(
        1 for b in nc.main_func.blocks for i in b.instructions
        if isinstance(__import__("concourse.mybir", fromlist=["x"]).InstLoadActFuncSet, type)
        and isinstance(i, __import__("concourse.mybir", fromlist=["x"]).InstLoadActFuncSet))
    if os.environ.get("NO_PRELOAD", "0") != "1":
        assert n_loads == 1, f"unexpected ACT table loads: {n_loads}"
    return nc, names


def prep_inputs(inputs, T=SEQ_LEN):
    """Host-side preprocessing: fold scales/signs into weights, precompute
    the x-projection, build constants. Returns in_map for the Bass kernel."""
    f = lambda k: np.asarray(inputs[k], dtype=np.float32)
    x = f("x")[:T]
    W_ih0, W_hh0 = f("W_ih0"), f("W_hh0")
    b_ih0, b_hh0 = f("b_ih0"), f("b_hh0")
    W_ih1, W_hh1 = f("W_ih1"), f("W_hh1")
    b_ih1, b_hh1 = f("b_ih1"), f("b_hh1")
    W_xi, b_xi = f("W_xi"), f("b_xi")
    W_out, b_out = f("W_out"), f("b_out")
    W_fc, b_fc = f("W_fc"), f("b_fc")

    gs = np.repeat(np.array([-1.0, -1.0, 2.0, -1.0], np.float32), H)  # (512,)

    xb = x @ W_ih0[:, :X_DIM].T + b_ih0 + b_hh0          # (T, 512)
    xb = xb * gs[None, :]
    xb3 = np.ascontiguousarray(
        xb.reshape(T, 4, H).transpose(2, 0, 1))          # (128, T, 4)

    w0ht = np.ascontiguousarray((W_hh0 * gs[:, None]).T)             # (128,512)
    w0rt = np.ascontiguousarray((W_ih0[:, X_DIM:] * gs[:, None]).T)  # (20,512)
    w1it = np.ascontiguousarray((W_ih1 * gs[:, None]).T)             # (128,512)
    w1ht = np.ascontiguousarray((W_hh1 * gs[:, None]).T)             # (128,512)
    b1 = np.ascontiguousarray(
        ((b_ih1 + b_hh1) * gs).reshape(4, H).T)                      # (128,4)

    # xi row reorder + scales
    idx = np.concatenate([
        np.arange(0, 20),        # rk   (tanh)
        np.arange(21, 41),       # wk   (tanh)
        np.arange(62, 82),       # wv   (tanh)
        np.arange(42, 62),       # er   (sigmoid)
        [20, 41],                # rb, wb (softplus)
        [82, 83, 84],            # fg, ga, gw (sigmoid)
    ]).astype(np.int64)
    sc = np.concatenate([
        2.0 * np.ones(60), -1.0 * np.ones(20), np.ones(2),
        [-1.0, -1.0, -1.0],
    ]).astype(np.float32)
    wxi_r = np.zeros((100, 128), np.float32)
    wxi_r[:85] = W_xi[idx] * sc[:, None]
    wxit = np.ascontiguousarray(wxi_r.T)                       # (128, 100)
    wpit = np.ascontiguousarray(W_xi[85:88].T)                 # (128, 3)
    assert np.abs(np.asarray(b_xi)).max() == 0.0, "kernel assumes b_xi == 0"

    Wcomb = W_fc @ W_out                                       # (10, 148)
    wct1 = np.ascontiguousarray(Wcomb[:, :H].T)                # (128, 10)
    wct2 = np.ascontiguousarray(Wcomb[:, H:].T)                # (20, 10)
    bcomb = (W_fc @ b_out + b_fc)[:, None].astype(np.float32)  # (10, 1)

    idn = np.eye(128, dtype=np.float32)
    jlt = np.tril(np.ones((10, 10), np.float32), -1)           # 1 if j<i ([i,j])
    diagm = (1.0 - np.eye(10)).astype(np.float32)

    ctrl_dt = (np.float16 if os.environ.get("CTRL_F16", "0") == "1"
               else np.float32)
    return {
        "xb3": xb3,
        "w0ht": w0ht.astype(ctrl_dt), "w0rt": w0rt.astype(ctrl_dt),
        "w1it": w1it.astype(ctrl_dt), "w1ht": w1ht.astype(ctrl_dt),
        "b1": b1, "wxit": wxit.astype(ctrl_dt), "wpit": wpit.astype(ctrl_dt),
        "wct1": wct1.astype(ctrl_dt), "wct2": wct2.astype(ctrl_dt),
        "bcomb": bcomb,
        "idn": idn, "jlt": jlt, "diagm": diagm,
    }


_BUILD_CACHE = {}


def kernel(**inputs):
    T = np.asarray(inputs["x"]).shape[0]
    key = (T, 8)
    if key not in _BUILD_CACHE:
        _BUILD_CACHE[key] = build(T=T, U=8)
    nc, names = _BUILD_CACHE[key]
    logical = prep_inputs(inputs, T=T)
    in_map = {names[k]: v for k, v in logical.items()}

    from concourse.bass_utils import run_bass_kernel_spmd
    res = run_bass_kernel_spmd(nc, [in_map], core_ids=[0])
    yt = res.results[0][names["yt"]]          # (10, T)
    return np.ascontiguousarray(yt.T)[None].astype(np.float32)


# revision 18
# speedup vs baseline: 1.0601x; 1.0601x over previous
"""DNC (Differentiable Neural Computer) sequential kernel for Trainium2.

Single-core Bass/Tile implementation: the 8192-step recurrence is strictly
sequential (sharding hint: unshardable within one sequence), so we run the
whole scan on core 0 with a hardware For_i loop, keeping all state in SBUF.

Key design points:
- ACT table set 6 (natural_log_exp_and_others) preloaded once; every
  transcendental (sigmoid/tanh/softplus/sqrt/softmax-exp) is expressed via
  exp/ln so no table switches ever happen.
- sigmoid(x) = 1/(1+e^-x); tanh(x) = 1-2/(1+e^{2x}); per-gate input signs
  and scales are folded into the weights on the host.
- x-projection of LSTM0 (W_ih0[:, :10] @ x_t) is precomputed on the host
  for all t (batched), streamed in per-chunk via DMA.
- The output projection is folded: y_t = (W_fc @ W_out) @ [h1; rv] + b,
  computed per-step on the PE and streamed out per-chunk.
- All tiny contractions / broadcasts / transposes run on the PE via
  matmuls with ones-vectors and identity; elementwise on DVE; exp/ln on ACT.
- Allocation weighting is sort-free: pairwise comparisons + masked cumprod
  (tensor_tensor_scan) reproduce the stable argsort semantics exactly.
"""

import sys
import os
import numpy as np

if "/opt/trn_rl_repo" not in sys.path:
    sys.path.insert(0, "/opt/trn_rl_repo")

# DNC hyperparameters (fixed by the problem)
N, CELL, R, H, X_DIM, OUT_DIM = 10, 20, 1, 128, 10, 10
IFACE = 88
EPS, DELTA = 1e-6, 1e-6
SEQ_LEN = 8192


def build(T=SEQ_LEN, U=8):
    """Build the Bass program. Returns (nc, names dict)."""
    import concourse.bass as bass
    import concourse.bacc as bacc
    from concourse import bass_isa
    import concourse.mybir as mybir
    from concourse import tile

    F32 = mybir.dt.float32
    F16 = mybir.dt.float16 if os.environ.get("CTRL_F16", "0") == "1" else mybir.dt.float32
    AF = mybir.ActivationFunctionType
    OP = mybir.AluOpType
    ds = bass.ds

    assert T % U == 0

    nc = bacc.Bacc(None, target_bir_lowering=False, debug=False)
    names = {}

    with tile.TileContext(nc) as tc:
        with tc.tile_pool(name="dram", bufs=1, space="DRAM") as dram, \
             tc.tile_pool(name="consts", bufs=1) as consts, \
             tc.tile_pool(name="state", bufs=1) as state, \
             tc.tile_pool(name="work", bufs=2) as work, \
             tc.tile_pool(name="io", bufs=2) as io, \
             tc.tile_pool(name="psG", bufs=2, space="PSUM") as psG, \
             tc.tile_pool(name="psX", bufs=1, space="PSUM") as psX, \
             tc.tile_pool(name="psA", bufs=1, space="PSUM") as psA, \
             tc.tile_pool(name="psB", bufs=1, space="PSUM") as psB, \
             tc.tile_pool(name="psC", bufs=1, space="PSUM") as psC:

            # ---------------- DRAM I/O ----------------
            d_xb3 = dram.tile([128, T, 4], F32, kind="ExternalInput")
            d_w0ht = dram.tile([128, 512], F16, kind="ExternalInput")
            d_w0rt = dram.tile([20, 512], F16, kind="ExternalInput")
            d_w1it = dram.tile([128, 512], F16, kind="ExternalInput")
            d_w1ht = dram.tile([128, 512], F16, kind="ExternalInput")
            d_b1 = dram.tile([128, 4], F32, kind="ExternalInput")
            d_wxit = dram.tile([128, 100], F16, kind="ExternalInput")
            d_wpit = dram.tile([128, 3], F16, kind="ExternalInput")
            d_wct1 = dram.tile([128, 10], F16, kind="ExternalInput")
            d_wct2 = dram.tile([20, 10], F16, kind="ExternalInput")
            d_bcomb = dram.tile([10, 1], F32, kind="ExternalInput")
            d_idn = dram.tile([128, 128], F32, kind="ExternalInput")
            d_jlt = dram.tile([10, 10], F32, kind="ExternalInput")
            d_diagm = dram.tile([10, 10], F32, kind="ExternalInput")
            d_yt = dram.tile([10, T], F32, kind="ExternalOutput")
            for k, v in [("xb3", d_xb3), ("w0ht", d_w0ht), ("w0rt", d_w0rt),
                         ("w1it", d_w1it), ("w1ht", d_w1ht), ("b1", d_b1),
                         ("wxit", d_wxit), ("wpit", d_wpit),
                         ("wct1", d_wct1), ("wct2", d_wct2), ("bcomb", d_bcomb),
                         ("idn", d_idn), ("jlt", d_jlt), ("diagm", d_diagm),
                         ("yt", d_yt)]:
                names[k] = v.tensor.name

            # ---- preload ACT function set 6 = natural_log_exp_and_others ----
            if os.environ.get("NO_PRELOAD", "0") != "1":
                nc.scalar.add_instruction(mybir.InstLoadActFuncSet(
                    name=nc.get_next_instruction_name(), act_func_set_id=6,
                    ins=[], outs=[]))

            # ---------------- const SBUF tiles ----------------
            W0hT = consts.tile([128, 512], F16)
            W0rT = consts.tile([20, 512], F16)
            W1iT = consts.tile([128, 512], F16)
            W1hT = consts.tile([128, 512], F16)
            B1 = consts.tile([128, 4], F32)
            WxiT = consts.tile([128, 100], F16)
            WpiT = consts.tile([128, 3], F16)
            WcT1 = consts.tile([128, 10], F16)
            WcT2 = consts.tile([20, 10], F16)
            BCOMB = consts.tile([10, 1], F32)
            IDN = consts.tile([128, 128], F32)
            JLT = consts.tile([10, 10], F32)
            DIAGM = consts.tile([10, 10], F32)
            ONESR = consts.tile([1, 32], F32)
            ONESC = consts.tile([32, 1], F32)
            ONES2D = consts.tile([10, 10], F32)

            for dst, src in [(W0hT, d_w0ht), (W0rT, d_w0rt), (W1iT, d_w1it),
                             (W1hT, d_w1ht), (B1, d_b1), (WxiT, d_wxit),
                             (WpiT, d_wpit), (WcT1, d_wct1),
                             (WcT2, d_wct2), (BCOMB, d_bcomb), (IDN, d_idn),
                             (JLT, d_jlt), (DIAGM, d_diagm)]:
                nc.sync.dma_start(dst[:], src[:])
            nc.vector.memset(ONESR[:], 1.0)
            nc.vector.memset(ONESC[:], 1.0)
            nc.vector.memset(ONES2D[:], 1.0)

            # ---------------- state tiles ----------------
            h0 = state.tile([128, 1], F16)
            c0 = state.tile([128, 1], F32)
            h1 = state.tile([128, 1], F16)
            c1 = state.tile([128, 1], F32)
            rv = state.tile([20, 1], F16)
            M32 = state.tile([32, 32], F32)
            Mt32 = state.tile([32, 32], F32)
            L32 = state.tile([32, 32], F32)
            LT32 = state.tile([32, 32], F32)
            M = M32[0:10, 0:20]
            Mt = Mt32[0:20, 0:10]
            L = L32[0:10, 0:10]
            LT = LT32[0:10, 0:10]
            negu = state.tile([1, 10], F32)     # -u
            p_row = state.tile([1, 10], F32)
            ww_row = state.tile([1, 10], F32)
            rw_row = state.tile([1, 10], F32)
            rw_col = state.tile([10, 1], F32)
            zM = state.tile([10, 1], F32)       # ||M_i||^2
            scr = state.tile([10, 20], F32)     # ttr dummy out

            for t in (h0, c0, h1, c1, rv, M32, Mt32, L32, LT32, negu,
                      p_row, ww_row, rw_row, rw_col, zM, scr):
                nc.vector.memset(t[:], 0.0)

            # =========================================================

            USE_GP = os.environ.get("NO_GPSIMD", "0") != "1"

            def bcast10(dst_sb, src_row_ap, ps_tile, ps_col):
                """broadcast [1,k]@p0 -> [10,k] SBUF tile"""
                if USE_GP:
                    nc.gpsimd.partition_broadcast(dst_sb[:], src_row_ap)
                else:
                    k = dst_sb.shape[1]
                    nc.tensor.matmul(ps_tile[0:10, ps_col:ps_col + k],
                                     lhsT=ONESR[:, 0:10], rhs=src_row_ap,
                                     start=True, stop=True)
                    nc.vector.tensor_copy(dst_sb[:],
                                          ps_tile[0:10, ps_col:ps_col + k])

            def sum10(dst_sb, src_col, ps_tile, ps_col):
                """dst[0:1,0:1] = sum over 10 partitions of src_col [10,1]"""
                if USE_GP:
                    nc.gpsimd.partition_all_reduce(
                        dst_sb[:], src_col[:], channels=10,
                        reduce_op=bass_isa.ReduceOp.add)
                else:
                    nc.tensor.matmul(ps_tile[0:1, ps_col:ps_col + 1],
                                     lhsT=src_col[:], rhs=ONESC[0:10, :],
                                     start=True, stop=True)
                    nc.vector.tensor_copy(dst_sb[0:1, 0:1],
                                          ps_tile[0:1, ps_col:ps_col + 1])
            def lstm_cell(G, WaT, rhs_a, WbT, rhs_b, xb_ap, c, h, tag):
                """gates psum <- sum of 8 matmuls; then exp-based cell."""
                for g in range(4):
                    nc.tensor.matmul(G[:, g:g + 1],
                                     lhsT=WaT[:, 128 * g:128 * (g + 1)],
                                     rhs=rhs_a[:], start=True, stop=False)
                    nc.tensor.matmul(G[:, g:g + 1],
                                     lhsT=WbT[:, 128 * g:128 * (g + 1)],
                                     rhs=rhs_b[:], start=False, stop=True)
                z = work.tile([128, 4], F32, tag=f"z{tag}")
                nc.vector.tensor_tensor(out=z[:], in0=G[:], in1=xb_ap,
                                        op=OP.add)
                E = work.tile([128, 4], F32, tag=f"E{tag}")
                nc.scalar.activation(E[:], z[:], AF.Exp)
                Rr = work.tile([128, 4], F32, tag=f"R{tag}")
                nc.vector.tensor_scalar(out=Rr[:], in0=E[:], scalar1=1.0,
                                        scalar2=None, op0=OP.add)
                nc.vector.reciprocal(Rr[:], Rr[:])
                m_ = work.tile([128, 1], F32, tag=f"m{tag}")
                nc.vector.tensor_tensor(out=m_[:], in0=Rr[:, 0:1],
                                        in1=Rr[:, 2:3], op=OP.mult)
                A_ = work.tile([128, 1], F32, tag=f"A{tag}")
                nc.vector.scalar_tensor_tensor(
                    out=A_[:], in0=m_[:], scalar=-2.0, in1=Rr[:, 0:1],
                    op0=OP.mult, op1=OP.add)
                nc.vector.scalar_tensor_tensor(
                    out=c[:], in0=c[:], scalar=Rr[:, 1:2], in1=A_[:],
                    op0=OP.mult, op1=OP.add)
                E2 = work.tile([128, 1], F32, tag=f"E2{tag}")
                nc.scalar.activation(E2[:], c[:], AF.Exp, scale=2.0)
                R2 = work.tile([128, 1], F32, tag=f"R2{tag}")
                nc.vector.tensor_scalar(out=R2[:], in0=E2[:], scalar1=1.0,
                                        scalar2=None, op0=OP.add)
                nc.vector.reciprocal(R2[:], R2[:])
                m2 = work.tile([128, 1], F32, tag=f"m2{tag}")
                nc.vector.tensor_tensor(out=m2[:], in0=Rr[:, 3:4], in1=R2[:],
                                        op=OP.mult)
                nc.vector.scalar_tensor_tensor(
                    out=h[:], in0=m2[:], scalar=-2.0, in1=Rr[:, 3:4],
                    op0=OP.mult, op1=OP.add)

            SKIP_MEM = os.environ.get("SKIP_MEM", "0") == "1"
            SKIP_CTRL = os.environ.get("SKIP_CTRL", "0") == "1"

            def step(XB, YT, u):
                # ---------- controller ----------
                if not SKIP_CTRL:
                    G0 = psG.tile([128, 4], F32, tag="G")
                    lstm_cell(G0, W0hT, h0, W0rT, rv, XB[:, u, :], c0, h0, "0")
                    G1 = psG.tile([128, 4], F32, tag="G")
                    lstm_cell(G1, W1iT, h0, W1hT, h1, B1[:], c1, h1, "1")

                # ---------- interface xi ----------
                # 5 segment matmuls, each landing at base partition 0 in its
                # own PSUM column: col0=rk col1=wk col2=wv col3=er
                # col4=[fg,ga,gw,rb,wb] (rows 0:5). pi -> row [1,3] cols 5:8.
                XIPI = psX.tile([32, 8], F32, tag="XIPI")
                for s in range(5):
                    nc.tensor.matmul(XIPI[0:20, s:s + 1],
                                     lhsT=WxiT[:, 20 * s:20 * (s + 1)],
                                     rhs=h1[:], start=True, stop=True)
                nc.tensor.matmul(XIPI[0:1, 5:8], lhsT=h1[:], rhs=WpiT[:],
                                 start=True, stop=True)
                # E = exp(scaled pre-acts); D = 1+E; R = 1/D
                DX = work.tile([20, 5], F32, tag="DX")
                nc.scalar.activation(DX[:], XIPI[0:20, 0:5], AF.Exp)
                nc.vector.tensor_scalar(out=DX[:], in0=DX[:], scalar1=1.0,
                                        scalar2=None, op0=OP.add)
                XIV = work.tile([20, 5], F32, tag="XIV")
                nc.vector.reciprocal(XIV[:], DX[:])
                # tanh groups (rk, wk, wv): t = 1-2R
                nc.vector.tensor_scalar(out=XIV[:, 0:3], in0=XIV[:, 0:3],
                                        scalar1=-2.0, scalar2=1.0,
                                        op0=OP.mult, op1=OP.add)
                # softplus rows (rb, wb) in col 4 rows 3:5: ln(1+e^x)
                nc.scalar.activation(XIV[0:2, 4:5], DX[0:2, 4:5], AF.Ln)
                # pi softmax (row form)
                EP = work.tile([1, 3], F32, tag="EP")
                sEP = work.tile([1, 1], F32, tag="sEP")
                nc.scalar.activation(EP[:], XIPI[0:1, 5:8], AF.Exp,
                                     accum_out=sEP[:])
                rsp = work.tile([1, 1], F32, tag="rsp")
                nc.vector.reciprocal(rsp[:], sEP[:])
                PI = work.tile([1, 3], F32, tag="PI")
                nc.vector.tensor_scalar(out=PI[:], in0=EP[:], scalar1=rsp[:],
                                        scalar2=None, op0=OP.mult)

                # transposes: wv col, er col, scal5 col -> one SBUF row
                spC = psC.tile([32, 256], F32, tag="spC")
                nc.tensor.transpose(spC[0:1, 0:20], XIV[:, 2:3],
                                    IDN[0:20, 0:20])
                nc.tensor.transpose(spC[0:1, 20:40], XIV[:, 3:4],
                                    IDN[0:20, 0:20])
                nc.tensor.transpose(spC[0:1, 40:45], XIV[0:5, 4:5],
                                    IDN[0:5, 0:5])
                ERWV = work.tile([1, 45], F32, tag="ERWV")
                nc.vector.tensor_copy(ERWV[:], spC[0:1, 0:45])
                # scal row = [rb, wb, fg, ga, gw]; ERWV[:, 0:40] = [wv | er]

                if SKIP_MEM:
                    nc.tensor.matmul(spC[0:10, 138:139], lhsT=WcT1[:],
                                     rhs=h1[:], start=True, stop=False)
                    nc.tensor.matmul(spC[0:10, 138:139], lhsT=WcT2[:],
                                     rhs=rv[:], start=False, stop=True)
                    nc.vector.tensor_scalar(out=YT[:, u:u + 1],
                                            in0=spC[0:10, 138:139],
                                            scalar1=BCOMB[:], scalar2=None,
                                            op0=OP.add)
                    return
                # ---------- usage / allocation ----------
                psiN = work.tile([1, 10], F32, tag="psiN")
                nc.vector.scalar_tensor_tensor(
                    out=psiN[:], in0=rw_row[:], scalar=ERWV[:, 42:43],
                    in1=ONESR[:, 0:10], op0=OP.mult, op1=OP.subtract)
                rr = work.tile([1, 10], F32, tag="rr")
                nc.vector.scalar_tensor_tensor(
                    out=rr[:], in0=negu[:], scalar=1.0, in1=ww_row[:],
                    op0=OP.add, op1=OP.mult)
                ss = work.tile([1, 10], F32, tag="ss")
                nc.vector.tensor_tensor(out=ss[:], in0=rr[:], in1=negu[:],
                                        op=OP.subtract)
                nc.vector.tensor_tensor(out=negu[:], in0=ss[:], in1=psiN[:],
                                        op=OP.mult)
                ue = work.tile([1, 10], F32, tag="ue")
                nc.vector.tensor_scalar(out=ue[:], in0=negu[:],
                                        scalar1=-(1.0 - DELTA), scalar2=DELTA,
                                        op0=OP.mult, op1=OP.add)

                spA = psA.tile([32, 256], F32, tag="spA")
                nc.tensor.transpose(spA[0:10, 0:1], ue[:], IDN[0:1, 0:1])
                uer = work.tile([10, 10], F32, tag="uer")
                nc.gpsimd.partition_broadcast(uer[:], ue[:])
                uec = work.tile([10, 1], F32, tag="uec")
                nc.vector.tensor_copy(uec[:], spA[0:10, 0:1])
                Bcmp = work.tile([10, 10], F32, tag="Bcmp")
                nc.vector.scalar_tensor_tensor(
                    out=Bcmp[:], in0=uer[:], scalar=uec[:],
                    in1=JLT[:], op0=OP.is_equal, op1=OP.mult)
                less = work.tile([10, 10], F32, tag="less")
                nc.vector.scalar_tensor_tensor(
                    out=less[:], in0=uer[:], scalar=uec[:],
                    in1=Bcmp[:], op0=OP.is_lt, op1=OP.add)
                UEm1 = work.tile([10, 10], F32, tag="UEm1")
                nc.vector.tensor_scalar(out=UEm1[:], in0=uer[:],
                                        scalar1=-1.0, scalar2=None, op0=OP.add)
                sel = work.tile([10, 10], F32, tag="sel")
                nc.vector.tensor_tensor(out=sel[:], in0=less[:], in1=UEm1[:],
                                        op=OP.mult)
                nc.vector.tensor_scalar(out=sel[:], in0=sel[:], scalar1=1.0,
                                        scalar2=None, op0=OP.add)
                cpv = work.tile([10, 10], F32, tag="cpv")
                nc.vector.tensor_tensor_scan(out=cpv[:], data0=sel[:],
                                             data1=ONES2D[:], initial=1.0,
                                             op0=OP.mult, op1=OP.mult)
                negalloc = work.tile([10, 1], F32, tag="negalloc")
                nc.vector.scalar_tensor_tensor(
                    out=negalloc[:], in0=uec[:], scalar=cpv[:, 9:10],
                    in1=cpv[:, 9:10], op0=OP.mult, op1=OP.subtract)

                # ---------- content weight (write key) ----------
                spB = psB.tile([32, 256], F32, tag="spB")
                nc.tensor.matmul(spB[0:10, 0:1], lhsT=Mt, rhs=XIV[0:20, 1:2],
                                 start=True, stop=True)
                nc.tensor.matmul(spB[0:1, 1:2], lhsT=XIV[0:20, 1:2],
                                 rhs=XIV[0:20, 1:2], start=True, stop=True)
                zks = work.tile([1, 1], F32, tag="zks")
                nc.vector.tensor_copy(zks[:], spB[0:1, 1:2])
                zkb = work.tile([10, 1], F32, tag="zkb")
                bcast10(zkb, zks[:], spB, 2)
                qw = work.tile([10, 1], F32, tag="qw")
                nc.vector.tensor_scalar(out=qw[:], in0=zM[:],
                                        scalar1=zkb[:], scalar2=1e-38,
                                        op0=OP.mult, op1=OP.max)
                nc.scalar.activation(qw[:], qw[:], AF.Ln)
                nc.scalar.activation(qw[:], qw[:], AF.Exp, scale=0.5)
                nc.vector.tensor_scalar(out=qw[:], in0=qw[:], scalar1=EPS,
                                        scalar2=None, op0=OP.add)
                rdd = work.tile([10, 1], F32, tag="rdd")
                nc.vector.reciprocal(rdd[:], qw[:])
                sim_ = work.tile([10, 1], F32, tag="sim_")
                nc.vector.tensor_tensor(out=sim_[:], in0=spB[0:10, 0:1],
                                        in1=rdd[:], op=OP.mult)
                bb = work.tile([10, 1], F32, tag="bb")
                bcast10(bb, ERWV[:, 41:42], spB, 3)
                bs = work.tile([10, 1], F32, tag="bs")
                nc.vector.tensor_tensor(out=bs[:], in0=sim_[:],
                                        in1=bb[:], op=OP.mult)
                ew = work.tile([10, 1], F32, tag="ew")
                nc.scalar.activation(ew[:], bs[:], AF.Exp)
                nc.tensor.matmul(spB[0:1, 4:5], lhsT=ew[:], rhs=ONESC[0:10, :],
                                 start=True, stop=True)
                rsw = work.tile([1, 1], F32, tag="rsw")
                nc.vector.reciprocal(rsw[:], spB[0:1, 4:5])
                nc.tensor.matmul(spB[0:10, 5:6], lhsT=ONESR[:, 0:10],
                                 rhs=rsw[:], start=True, stop=True)


                # ---------- write weights ----------
                mga = work.tile([1, 1], F32, tag="mga")
                nc.vector.tensor_tensor(out=mga[:], in0=ERWV[:, 43:44],
                                        in1=ERWV[:, 44:45], op=OP.mult)
                CO = work.tile([1, 2], F32, tag="CO")
                nc.vector.tensor_scalar(out=CO[:, 0:1], in0=mga[:],
                                        scalar1=-1.0, scalar2=None,
                                        op0=OP.mult)
                nc.vector.scalar_tensor_tensor(
                    out=CO[:, 1:2], in0=mga[:], scalar=-1.0, in1=ERWV[:, 44:45],
                    op0=OP.mult, op1=OP.add)
                cob = work.tile([10, 2], F32, tag="cob")
                bcast10(cob, CO[:], spB, 6)
                t2 = work.tile([10, 1], F32, tag="t2")
                nc.vector.scalar_tensor_tensor(
                    out=t2[:], in0=spB[0:10, 5:6], scalar=cob[:, 1:2],
                    in1=ew[:], op0=OP.mult, op1=OP.mult)
                wwc = work.tile([10, 1], F32, tag="wwc")
                nc.vector.scalar_tensor_tensor(
                    out=wwc[:], in0=negalloc[:], scalar=cob[:, 0:1],
                    in1=t2[:], op0=OP.mult, op1=OP.add)
                nc.tensor.transpose(spB[0:1, 8:18], wwc[:], IDN[0:10, 0:10])
                nc.vector.tensor_copy(ww_row[:], spB[0:1, 8:18])

                # ---------- memory update ----------
                erwvb = work.tile([10, 40], F32, tag="erwvb")
                nc.gpsimd.partition_broadcast(erwvb[:], ERWV[:, 0:40])
                m1 = work.tile([10, 20], F32, tag="m1")
                nc.vector.scalar_tensor_tensor(
                    out=m1[:], in0=erwvb[:, 20:40], scalar=wwc[:], in1=M,
                    op0=OP.mult, op1=OP.mult)
                M2 = work.tile([10, 20], F32, tag="M2")
                nc.vector.tensor_tensor(out=M2[:], in0=M, in1=m1[:],
                                        op=OP.subtract)
                nc.vector.scalar_tensor_tensor(
                    out=M, in0=erwvb[:, 0:20], scalar=wwc[:], in1=M2[:],
                    op0=OP.mult, op1=OP.add)
                nc.vector.scalar_tensor_tensor(
                    out=scr[:], in0=M, scalar=1.0, in1=M,
                    op0=OP.mult, op1=OP.mult, accum_out=zM[:])
                nc.vector.transpose(Mt32[:], M32[:])

                # ---------- link matrix ----------
                wwr2 = work.tile([10, 10], F32, tag="wwr2")
                nc.gpsimd.partition_broadcast(wwr2[:], ww_row[:])
                pr2 = work.tile([10, 10], F32, tag="pr2")
                nc.gpsimd.partition_broadcast(pr2[:], p_row[:])
                S_ = work.tile([10, 10], F32, tag="S_")
                nc.vector.scalar_tensor_tensor(
                    out=S_[:], in0=wwr2[:], scalar=wwc[:], in1=L,
                    op0=OP.add, op1=OP.mult)
                LmS = work.tile([10, 10], F32, tag="LmS")
                nc.vector.tensor_tensor(out=LmS[:], in0=L, in1=S_[:],
                                        op=OP.subtract)
                Ln_ = work.tile([10, 10], F32, tag="Ln_")
                nc.vector.scalar_tensor_tensor(
                    out=Ln_[:], in0=pr2[:], scalar=wwc[:],
                    in1=LmS[:], op0=OP.mult, op1=OP.add)
                nc.vector.tensor_tensor(out=L, in0=Ln_[:], in1=DIAGM[:],
                                        op=OP.mult)
                nc.vector.transpose(LT32[:], L32[:])

                # ---------- precedence ----------
                swp = work.tile([10, 1], F32, tag="swp")
                sum10(swp, wwc, spC, 115)
                tp = work.tile([1, 10], F32, tag="tp")
                nc.vector.scalar_tensor_tensor(
                    out=tp[:], in0=p_row[:], scalar=swp[0:1, 0:1],
                    in1=ww_row[:], op0=OP.mult, op1=OP.subtract)
                nc.vector.tensor_tensor(out=p_row[:], in0=p_row[:], in1=tp[:],
                                        op=OP.subtract)

                # ---------- read ----------
                nc.tensor.matmul(spC[0:1, 116:126], lhsT=rw_col[:], rhs=L,
                                 start=True, stop=True)     # bwd
                nc.tensor.matmul(spC[0:1, 126:136], lhsT=rw_col[:], rhs=LT,
                                 start=True, stop=True)     # fwd
                nc.tensor.matmul(spB[0:10, 20:21], lhsT=Mt, rhs=XIV[0:20, 0:1],
                                 start=True, stop=True)     # dotr
                nc.tensor.matmul(spB[0:1, 21:22], lhsT=XIV[0:20, 0:1],
                                 rhs=XIV[0:20, 0:1], start=True, stop=True)
                zkrs = work.tile([1, 1], F32, tag="zkrs")
                nc.vector.tensor_copy(zkrs[:], spB[0:1, 21:22])
                zkrb = work.tile([10, 1], F32, tag="zkrb")
                bcast10(zkrb, zkrs[:], spB, 22)
                qr = work.tile([10, 1], F32, tag="qr")
                nc.vector.tensor_scalar(out=qr[:], in0=zM[:],
                                        scalar1=zkrb[:],
                                        scalar2=1e-38, op0=OP.mult, op1=OP.max)
                nc.scalar.activation(qr[:], qr[:], AF.Ln)
                nc.scalar.activation(qr[:], qr[:], AF.Exp, scale=0.5)
                nc.vector.tensor_scalar(out=qr[:], in0=qr[:], scalar1=EPS,
                                        scalar2=None, op0=OP.add)
                rdr = work.tile([10, 1], F32, tag="rdr")
                nc.vector.reciprocal(rdr[:], qr[:])
                simr = work.tile([10, 1], F32, tag="simr")
                nc.vector.tensor_tensor(out=simr[:], in0=spB[0:10, 20:21],
                                        in1=rdr[:], op=OP.mult)
                rbb = work.tile([10, 1], F32, tag="rbb")
                bcast10(rbb, ERWV[:, 40:41], spB, 23)
                bsr = work.tile([10, 1], F32, tag="bsr")
                nc.vector.tensor_tensor(out=bsr[:], in0=simr[:],
                                        in1=rbb[:], op=OP.mult)
                er_ = work.tile([10, 1], F32, tag="er_")
                nc.scalar.activation(er_[:], bsr[:], AF.Exp)
                nc.tensor.matmul(spB[0:1, 24:25], lhsT=er_[:],
                                 rhs=ONESC[0:10, :], start=True, stop=True)
                rswr = work.tile([1, 1], F32, tag="rswr")
                nc.vector.reciprocal(rswr[:], spB[0:1, 24:25])
                pi1n = work.tile([1, 1], F32, tag="pi1n")
                nc.vector.tensor_tensor(out=pi1n[:], in0=PI[:, 1:2],
                                        in1=rswr[:], op=OP.mult)
                nc.tensor.transpose(spB[0:1, 26:36], er_[:], IDN[0:10, 0:10])

                Bf = work.tile([1, 10], F32, tag="Bf")
                nc.vector.tensor_scalar(out=Bf[:], in0=spC[0:1, 126:136],
                                        scalar1=PI[:, 2:3], scalar2=None,
                                        op0=OP.mult)
                Af = work.tile([1, 10], F32, tag="Af")
                nc.vector.scalar_tensor_tensor(
                    out=Af[:], in0=spC[0:1, 116:126], scalar=PI[:, 0:1],
                    in1=Bf[:], op0=OP.mult, op1=OP.add)
                nc.vector.scalar_tensor_tensor(
                    out=rw_row[:], in0=spB[0:1, 26:36], scalar=pi1n[:],
                    in1=Af[:], op0=OP.mult, op1=OP.add)
                nc.tensor.transpose(spC[0:10, 136:137], rw_row[:],
                                    IDN[0:1, 0:1])
                nc.vector.tensor_copy(rw_col[:], spC[0:10, 136:137])
                nc.tensor.matmul(spC[0:20, 137:138], lhsT=M, rhs=rw_col[:],
                                 start=True, stop=True)
                nc.vector.tensor_copy(rv[:], spC[0:20, 137:138])

                # ---------- output ----------
                nc.tensor.matmul(spC[0:10, 138:139], lhsT=WcT1[:], rhs=h1[:],
                                 start=True, stop=False)
                nc.tensor.matmul(spC[0:10, 138:139], lhsT=WcT2[:], rhs=rv[:],
                                 start=False, stop=True)
                nc.vector.tensor_scalar(out=YT[:, u:u + 1],
                                        in0=spC[0:10, 138:139],
                                        scalar1=BCOMB[:], scalar2=None,
                                        op0=OP.add)

            # ================= main loop =================
            import concourse.mybir as _mb
            _hints = () if os.environ.get("NO_HINTS", "0") == "1" else tuple(_mb.ALL_ENGINES)
            _stag = os.environ.get("STAGGERED", "1") == "1"
            with tc.For_i(0, T, U, hint_engines=_hints,
                          staggered_reset=_stag) as iv:
                XB = io.tile([128, U, 4], F32, tag="XB")
                nc.sync.dma_start(XB[:], d_xb3[:, ds(iv, U), :])
                YT = io.tile([10, U], F32, tag="YT")
                for u in range(U):
                    step(XB, YT, u)
                nc.sync.dma_start(d_yt[:, ds(iv, U)], YT[:])

    nc.compile()

    # sanity: exactly one ACT table load (the manual preload)
    n_loads = sum(
        1 for b in nc.main_func.blocks for i in b.instructions
        if isinstance(__import__("concourse.mybir", fromlist=["x"]).InstLoadActFuncSet, type)
        and isinstance(i, __import__("concourse.mybir", fromlist=["x"]).InstLoadActFuncSet))
    if os.environ.get("NO_PRELOAD", "0") != "1":
        assert n_loads == 1, f"unexpected ACT table loads: {n_loads}"
    return nc, names


def prep_inputs(inputs, T=SEQ_LEN):
    """Host-side preprocessing: fold scales/signs into weights, precompute
    the x-projection, build constants. Returns in_map for the Bass kernel."""
    f = lambda k: np.asarray(inputs[k], dtype=np.float32)
    x = f("x")[:T]
    W_ih0, W_hh0 = f("W_ih0"), f("W_hh0")
    b_ih0, b_hh0 = f("b_ih0"), f("b_hh0")
    W_ih1, W_hh1 = f("W_ih1"), f("W_hh1")
    b_ih1, b_hh1 = f("b_ih1"), f("b_hh1")
    W_xi, b_xi = f("W_xi"), f("b_xi")
    W_out, b_out = f("W_out"), f("b_out")
    W_fc, b_fc = f("W_fc"), f("b_fc")

    gs = np.repeat(np.array([-1.0, -1.0, 2.0, -1.0], np.float32), H)  # (512,)

    xb = x @ W_ih0[:, :X_DIM].T + b_ih0 + b_hh0          # (T, 512)
    xb = xb * gs[None, :]
    xb3 = np.ascontiguousarray(
        xb.reshape(T, 4, H).transpose(2, 0, 1))          # (128, T, 4)

    w0ht = np.ascontiguousarray((W_hh0 * gs[:, None]).T)             # (128,512)
    w0rt = np.ascontiguousarray((W_ih0[:, X_DIM:] * gs[:, None]).T)  # (20,512)
    w1it = np.ascontiguousarray((W_ih1 * gs[:, None]).T)             # (128,512)
    w1ht = np.ascontiguousarray((W_hh1 * gs[:, None]).T)             # (128,512)
    b1 = np.ascontiguousarray(
        ((b_ih1 + b_hh1) * gs).reshape(4, H).T)                      # (128,4)

    # xi row reorder + scales
    idx = np.concatenate([
        np.arange(0, 20),        # rk   (tanh)
        np.arange(21, 41),       # wk   (tanh)
        np.arange(62, 82),       # wv   (tanh)
        np.arange(42, 62),       # er   (sigmoid)
        [20, 41],                # rb, wb (softplus)
        [82, 83, 84],            # fg, ga, gw (sigmoid)
    ]).astype(np.int64)
    sc = np.concatenate([
        2.0 * np.ones(60), -1.0 * np.ones(20), np.ones(2),
        [-1.0, -1.0, -1.0],
    ]).astype(np.float32)
    wxi_r = np.zeros((100, 128), np.float32)
    wxi_r[:85] = W_xi[idx] * sc[:, None]
    wxit = np.ascontiguousarray(wxi_r.T)                       # (128, 100)
    wpit = np.ascontiguousarray(W_xi[85:88].T)                 # (128, 3)
    assert np.abs(np.asarray(b_xi)).max() == 0.0, "kernel assumes b_xi == 0"

    Wcomb = W_fc @ W_out                                       # (10, 148)
    wct1 = np.ascontiguousarray(Wcomb[:, :H].T)                # (128, 10)
    wct2 = np.ascontiguousarray(Wcomb[:, H:].T)                # (20, 10)
    bcomb = (W_fc @ b_out + b_fc)[:, None].astype(np.float32)  # (10, 1)

    idn = np.eye(128, dtype=np.float32)
    jlt = np.tril(np.ones((10, 10), np.float32), -1)           # 1 if j<i ([i,j])
    diagm = (1.0 - np.eye(10)).astype(np.float32)

    ctrl_dt = (np.float16 if os.environ.get("CTRL_F16", "0") == "1"
               else np.float32)
    return {
        "xb3": xb3,
        "w0ht": w0ht.astype(ctrl_dt), "w0rt": w0rt.astype(ctrl_dt),
        "w1it": w1it.astype(ctrl_dt), "w1ht": w1ht.astype(ctrl_dt),
        "b1": b1, "wxit": wxit.astype(ctrl_dt), "wpit": wpit.astype(ctrl_dt),
        "wct1": wct1.astype(ctrl_dt), "wct2": wct2.astype(ctrl_dt),
        "bcomb": bcomb,
        "idn": idn, "jlt": jlt, "diagm": diagm,
    }


_BUILD_CACHE = {}


def kernel(**inputs):
    T = np.asarray(inputs["x"]).shape[0]
    key = (T, 4)
    if key not in _BUILD_CACHE:
        _BUILD_CACHE[key] = build(T=T, U=4)
    nc, names = _BUILD_CACHE[key]
    logical = prep_inputs(inputs, T=T)
    in_map = {names[k]: v for k, v in logical.items()}

    from concourse.bass_utils import run_bass_kernel_spmd
    res = run_bass_kernel_spmd(nc, [in_map], core_ids=[0])
    yt = res.results[0][names["yt"]]          # (10, T)
    return np.ascontiguousarray(yt.T)[None].astype(np.float32)
